# revision 56
# baseline (speedup 1.0000x reference)
"""ContrastiveLoss Trainium2 kernel (v2).

Contract: kernel(feature, label) -> (loss, mean_pos, mean_neg), matching
the reference. Full inputs in, full outputs out; internally sharded across
8 NeuronCores (each core owns 1024 rows of z and computes its [1024, 8192]
similarity slab).

Host prep (per the sharding hint, devices receive z): sort rows by label
(1s first), L2-normalize, pre-transpose z^T; bf16 copy for the sim matmuls,
fp32 local slice + [S, S1] column sums for the exact mean_pos/mean_neg path.

Device per core:
  - 64 bf16 matmuls (N=1024) build the 1024x8192 sim slab in PSUM chunks.
  - exp(2*sim) row-sums split between ScalarE (exact ACT exp + accum, first
    W_S cols of each 2048 chunk) and VectorE (Schraudolph bit-trick exp:
    one affine->int32 op + reduce over the fp32-bitcast, remaining cols).
    Label-sorted columns make the masked sums range splits.
  - 8 tiny fp32 matmuls z_loc @ [S, S1] give the mean accumulators.
Host combine: logs for the per-row loss (shipped as num/den sums), scalar
reductions across cores.
"""

import sys

sys.path.insert(0, "/opt/trn_rl_repo")

import numpy as np
import ml_dtypes

import concourse.bass as bass
import concourse.mybir as mybir
import concourse.tile as tile
from concourse import bacc
from concourse.bass import ds, ts
from concourse.bass_utils import run_bass_kernel_spmd

B = 8192
D = 128
N_CORES = 8
LOCAL = B // N_CORES          # 1024 rows per core
TILES_I = LOCAL // 128        # 8 local row tiles per core
CHUNK = 2048                  # psum chunk width (4 banks)
NCHUNK = B // CHUNK
W_S = 1376                    # scalar-engine exp columns per chunk


def _ws_by_chunk(n1):
    """Per-chunk scalar width: the boundary chunk pays an extra vector-side
    accumulate, so it gets a wider scalar range to stay balanced."""
    ws = [W_S] * NCHUNK
    for q in range(NCHUNK):
        nl = n1 - q * CHUNK
        if 0 < nl < CHUNK and nl < CHUNK - (W_S + 64):
            ws[q] = W_S + 64
    return ws
EPS = 1e-8
EII = float(np.exp(2.0))      # exp(2*sim_ii), sim_ii == 1 after normalize

LOG2E = 1.4426950408889634
SCHRA_A = float(np.float32(2.0 * LOG2E * (1 << 23)))
SCHRA_C = 298765
SCHRA_B = float(np.float32((127 << 23) - SCHRA_C))

FP32 = mybir.dt.float32
BF16 = mybir.dt.bfloat16
INT32 = mybir.dt.int32
AF = mybir.ActivationFunctionType
ALU = mybir.AluOpType
AX = mybir.AxisListType


def _segments(n1):
    """Per-chunk exp segments: (lo, hi, is_label1, engine) with engine
    's' for ScalarE (W_S cols of each chunk) or 'v' for VectorE. The
    scalar range sits at the chunk start, except in the chunk containing
    the label boundary where it is placed to keep the split on the
    (cheaper) vector side when possible."""
    wsl = _ws_by_chunk(n1)
    segs = []
    for q in range(NCHUNK):
        ws = wsl[q]
        c0, c1 = q * CHUNK, (q + 1) * CHUNK
        nl = n1 - c0
        if 0 < nl < CHUNK and nl < CHUNK - ws:
            ranges = ((c1 - ws, c1, 's'), (c0, c1 - ws, 'v'))
        else:
            ranges = ((c0, c0 + ws, 's'), (c0 + ws, c1, 'v'))
        for lo, hi, eng in ranges:
            if n1 <= lo:
                segs.append((lo, hi, False, eng))
            elif n1 >= hi:
                segs.append((lo, hi, True, eng))
            else:
                segs.append((lo, n1, True, eng))
                segs.append((n1, hi, False, eng))
    # label-1 segments first so s1/s0 are contiguous reduces over sacc
    segs = [s for s in segs if s[2]] + [s for s in segs if not s[2]]
    return segs


def _build_kernel(n1: int):
    nc = bacc.Bacc("TRN2", target_bir_lowering=False, debug=False,
                   num_devices=N_CORES)
    ztb = nc.dram_tensor("ztb", [128, B], BF16, kind="ExternalInput").ap()
    zltb = nc.dram_tensor("zltb", [128, LOCAL], BF16,
                          kind="ExternalInput").ap()
    zl = nc.dram_tensor("zl", [128, LOCAL], FP32, kind="ExternalInput").ap()
    svec = nc.dram_tensor("svec", [128, 2], FP32, kind="ExternalInput").ap()
    outp = nc.dram_tensor("outp", [128, 4 * TILES_I], FP32,
                          kind="ExternalOutput").ap()

    segs = _segments(n1)
    nseg = len(segs)
    cb = sum(1 for s in segs if s[2])
    W_V = CHUNK - W_S

    with tile.TileContext(nc) as tc:
        with (
            tc.tile_pool(name="zp", bufs=1) as zp,
            tc.tile_pool(name="small", bufs=1) as small,
            tc.tile_pool(name="scr", bufs=4) as scrp,
            tc.tile_pool(name="psum", bufs=2, space=bass.MemorySpace.PSUM) as psum,
        ):
            # hoist the Exp table load into the DMA window: a dummy tiny
            # activation first in the scalar queue triggers the (2.7us)
            # ACT_TABLE_LOAD before any real work needs it
            dummy = small.tile([128, 1], FP32, tag="dummy")
            nc.vector.memset(dummy[:], 0.0)
            nc.scalar.activation(dummy[:], dummy[:], AF.Exp)

            # trigger order = HWDGE queue order: svec-path inputs first (they
            # gate the small head-of-queue PE work), then chunk 0, then rest
            zlt = small.tile([128, LOCAL], FP32, tag="zlt")
            nc.sync.dma_start(zlt[:], zl)
            sv = small.tile([128, 2], FP32, tag="sv")
            nc.sync.dma_start(sv[:], svec)
            zlb = small.tile([128, LOCAL], BF16, tag="zlb")
            nc.sync.dma_start(zlb[:], zltb)
            zt = zp.tile([128, B], BF16, tag="zt")
            nc.sync.dma_start(zt[:, 0:CHUNK], ztb[:, 0:CHUNK])
            for q in range(1, NCHUNK):
                for h in range(2):
                    nc.sync.dma_start(
                        zt[:, q * CHUNK + h * 1024:q * CHUNK + (h + 1) * 1024],
                        ztb[:, q * CHUNK + h * 1024:q * CHUNK + (h + 1) * 1024])

            sacc = small.tile([128, TILES_I, nseg], FP32, tag="sacc")
            srr = small.tile([128, 2 * TILES_I], FP32, tag="srr")
            s1r = srr[:, 0:TILES_I]
            s0r = srr[:, TILES_I:2 * TILES_I]

            def chunk_iters(q):
                """One sim chunk: matmuls + split exp row sums, all 8 tiles."""
                qsegs = [(i, s) for i, s in enumerate(segs)
                         if q * CHUNK <= s[0] and s[1] <= (q + 1) * CHUNK]
                v_lo = min(s[0] for _, s in qsegs if s[3] == 'v')
                w_v = CHUNK - _ws_by_chunk(n1)[q]
                for t in range(TILES_I):
                    ps = psum.tile([128, CHUNK], FP32, tag="ps")
                    for h in range(4):
                        nc.tensor.matmul(
                            ps[:, ts(h, 512)], lhsT=zlb[:, ts(t, 128)],
                            rhs=zt[:, q * CHUNK + h * 512:
                                   q * CHUNK + (h + 1) * 512],
                            start=True, stop=True)
                    scr = scrp.tile([128, W_V], INT32, tag="scr")
                    scrf = scr.bitcast(FP32)
                    dead = scrp.tile([128, W_V], FP32, tag="dead")
                    sdead = scrp.tile([128, W_S + 64], BF16, tag="sdead")
                    for si, (lo, hi, _l1, eng) in qsegs:
                        if eng == 's':
                            nc.scalar.activation(
                                sdead[:, 0:hi - lo],
                                ps[:, lo - q * CHUNK:hi - q * CHUNK],
                                AF.Exp, scale=2.0,
                                accum_out=sacc[:, t, si:si + 1])
                    # one affine->int32 pass for the whole DVE range
                    nc.vector.tensor_scalar(
                        out=scr[:, 0:w_v],
                        in0=ps[:, v_lo - q * CHUNK:v_lo - q * CHUNK + w_v],
                        scalar1=SCHRA_A, scalar2=SCHRA_B,
                        op0=ALU.mult, op1=ALU.add)
                    # row sums via tensor_scalar accum
                    for si, (lo, hi, _l1, eng) in qsegs:
                        if eng == 'v':
                            nc.vector.tensor_scalar(
                                out=dead[:, 0:hi - lo],
                                in0=scrf[:, lo - v_lo:hi - v_lo],
                                scalar1=1.0, scalar2=None, op0=ALU.mult,
                                op1=ALU.add,
                                accum_out=sacc[:, t, si:si + 1])

            # tall/t1 = z_loc @ [S, S1] (fp32 exact path for the means) —
            # head of the PE queue, runs during the input DMA window; the
            # raw [tall|t1] block ships to the host mid-loop (the remaining
            # per-row algebra is done host-side in fp64)
            pt = psum.tile([128, CHUNK], FP32, tag="ps")
            for t in range(TILES_I):
                nc.tensor.matmul(pt[:, 2 * t:2 * t + 2],
                                 lhsT=zlt[:, ts(t, 128)], rhs=sv[:],
                                 start=True, stop=True)
            tt = small.tile([128, 2 * TILES_I], FP32, tag="tt")
            nc.vector.tensor_copy(tt[:], pt[:, 0:2 * TILES_I])
            nc.sync.dma_start(outp[:, 0:2 * TILES_I], tt[:])

            chunk_iters(0)

            # remaining chunks; hoist the s1 reduce to right after the last
            # chunk containing label-1 columns so only s0 work is in the tail
            q_l1 = max((s[1] - 1) // CHUNK for s in segs if s[2]) \
                if cb > 0 else -1
            for q in range(1, NCHUNK):
                chunk_iters(q)
                if q == q_l1:
                    nc.vector.reduce_sum(s1r, sacc[:, :, 0:cb], axis=AX.X)
            if q_l1 <= 0:
                if cb > 0:
                    nc.vector.reduce_sum(s1r, sacc[:, :, 0:cb], axis=AX.X)
                else:
                    nc.vector.memset(s1r, 0.0)

            # ---- tail: just the label-0 reduce + raw row-sum shipment ----
            if cb < nseg:
                nc.vector.reduce_sum(s0r, sacc[:, :, cb:nseg], axis=AX.X)
            else:
                nc.vector.memset(s0r, 0.0)
            nc.sync.dma_start(outp[:, 2 * TILES_I:4 * TILES_I], srr[:])

    nc.compile()
    return nc


_NC_CACHE = {}


def _get_nc(n1: int = 4083):
    if n1 not in _NC_CACHE:
        _NC_CACHE[n1] = _build_kernel(n1)
    return _NC_CACHE[n1]


def prepare(feature: np.ndarray, label: np.ndarray):
    """Sort rows by label (1s first), normalize, transpose; per-core maps."""
    feature = np.ascontiguousarray(feature, dtype=np.float32)
    lab = np.asarray(label)
    perm = np.argsort(-lab, kind="stable")
    n1 = int((lab == 1).sum())
    f = feature[perm]
    nrm = np.maximum(np.sqrt((f.astype(np.float64) ** 2).sum(1)), 1e-12)
    z = (f / nrm[:, None]).astype(np.float32)
    zT = np.ascontiguousarray(z.T)                    # [128, B] fp32
    ztb = zT.astype(ml_dtypes.bfloat16)               # [128, B] bf16
    zf = z.astype(np.float64)
    S = zf.sum(0)
    S1 = zf[:n1].sum(0)
    sv = np.ascontiguousarray(
        np.stack([S, S1], axis=1).astype(np.float32))  # [128, 2]
    lsort = lab[perm].astype(np.float64)
    # per-core label matrix in the device's [128, TILES_I] layout
    labm = lsort.reshape(N_CORES, TILES_I, 128).transpose(0, 2, 1)
    in_maps = []
    for c in range(N_CORES):
        sl = slice(c * LOCAL, (c + 1) * LOCAL)
        in_maps.append({
            "ztb": ztb,
            "zltb": np.ascontiguousarray(ztb[:, sl]),
            "zl": np.ascontiguousarray(zT[:, sl]),
            "svec": sv,
        })
    return n1, in_maps, labm


def combine(results, labm):
    """outp = [tall|t1 interleaved (16) | s1r (8) | s0r (8)] per core;
    final per-row algebra and the logs run here in fp64."""
    P = np.stack([np.asarray(r["outp"], dtype=np.float64) for r in results])
    tall = P[:, :, 0:2 * TILES_I:2]
    t1 = P[:, :, 1:2 * TILES_I:2]
    s1 = P[:, :, 2 * TILES_I:3 * TILES_I]
    s0 = P[:, :, 3 * TILES_I:4 * TILES_I]
    pos = labm > 0.5
    same = np.where(pos, s1, s0)
    num = same - EII
    dennum = s1 + s0 - EII
    loss = (np.log(dennum) - np.log(np.maximum(num, 0.0) + EPS)).sum() / B
    same_t = np.where(pos, t1, tall - t1)
    mean_pos = (same_t - 1.0).sum() / (float(B) * B)
    mean_neg = (tall - same_t).sum() / (float(B) * B)
    return (np.float32(loss), np.float32(mean_pos), np.float32(mean_neg))


def run_on_hw(feature, label, **kwargs):
    n1, in_maps, labm = prepare(feature, label)
    nc = _get_nc(n1)
    res = run_bass_kernel_spmd(nc, in_maps,
                               core_ids=list(range(N_CORES)), **kwargs)
    return combine(res.results, labm), res


def kernel(feature: np.ndarray, label: np.ndarray):
    out, _ = run_on_hw(feature, label)
    return out


# revision 57
# speedup vs baseline: 1.0122x; 1.0122x over previous
"""ContrastiveLoss Trainium2 kernel (v2).

Contract: kernel(feature, label) -> (loss, mean_pos, mean_neg), matching
the reference. Full inputs in, full outputs out; internally sharded across
8 NeuronCores (each core owns 1024 rows of z and computes its [1024, 8192]
similarity slab).

Host prep (per the sharding hint, devices receive z): sort rows by label
(1s first), L2-normalize, pre-transpose z^T; bf16 copy for the sim matmuls,
fp32 local slice + [S, S1] column sums for the exact mean_pos/mean_neg path.

Device per core:
  - 64 bf16 matmuls (N=1024) build the 1024x8192 sim slab in PSUM chunks.
  - exp(2*sim) row-sums split between ScalarE (exact ACT exp + accum, first
    W_S cols of each 2048 chunk) and VectorE (Schraudolph bit-trick exp:
    one affine->int32 op + reduce over the fp32-bitcast, remaining cols).
    Label-sorted columns make the masked sums range splits.
  - 8 tiny fp32 matmuls z_loc @ [S, S1] give the mean accumulators.
Host combine: logs for the per-row loss (shipped as num/den sums), scalar
reductions across cores.
"""

import sys

sys.path.insert(0, "/opt/trn_rl_repo")

import numpy as np
import ml_dtypes

import concourse.bass as bass
import concourse.mybir as mybir
import concourse.tile as tile
from concourse import bacc
from concourse.bass import ds, ts
from concourse.bass_utils import run_bass_kernel_spmd

B = 8192
D = 128
N_CORES = 8
LOCAL = B // N_CORES          # 1024 rows per core
TILES_I = LOCAL // 128        # 8 local row tiles per core
CHUNK = 2048                  # psum chunk width (4 banks)
NCHUNK = B // CHUNK
W_S = 1376                    # scalar-engine exp columns per chunk


def _ws_by_chunk(n1):
    """Per-chunk scalar-engine exp width (uniform; a wider scalar range on
    the label-boundary chunk measured slower on HW despite the extra
    vector-side accumulate there)."""
    return [W_S] * NCHUNK


EPS = 1e-8
EII = float(np.exp(2.0))      # exp(2*sim_ii), sim_ii == 1 after normalize

LOG2E = 1.4426950408889634
SCHRA_A = float(np.float32(2.0 * LOG2E * (1 << 23)))
SCHRA_C = 298765
SCHRA_B = float(np.float32((127 << 23) - SCHRA_C))

FP32 = mybir.dt.float32
BF16 = mybir.dt.bfloat16
INT32 = mybir.dt.int32
AF = mybir.ActivationFunctionType
ALU = mybir.AluOpType
AX = mybir.AxisListType


def _segments(n1):
    """Per-chunk exp segments: (lo, hi, is_label1, engine) with engine
    's' for ScalarE (W_S cols of each chunk) or 'v' for VectorE. The
    scalar range sits at the chunk start, except in the chunk containing
    the label boundary where it is placed to keep the split on the
    (cheaper) vector side when possible."""
    wsl = _ws_by_chunk(n1)
    segs = []
    for q in range(NCHUNK):
        ws = wsl[q]
        c0, c1 = q * CHUNK, (q + 1) * CHUNK
        nl = n1 - c0
        if 0 < nl < CHUNK and nl < CHUNK - ws:
            ranges = ((c1 - ws, c1, 's'), (c0, c1 - ws, 'v'))
        else:
            ranges = ((c0, c0 + ws, 's'), (c0 + ws, c1, 'v'))
        for lo, hi, eng in ranges:
            if n1 <= lo:
                segs.append((lo, hi, False, eng))
            elif n1 >= hi:
                segs.append((lo, hi, True, eng))
            else:
                segs.append((lo, n1, True, eng))
                segs.append((n1, hi, False, eng))
    # label-1 segments first so s1/s0 are contiguous reduces over sacc
    segs = [s for s in segs if s[2]] + [s for s in segs if not s[2]]
    return segs


def _build_kernel(n1: int):
    nc = bacc.Bacc("TRN2", target_bir_lowering=False, debug=False,
                   num_devices=N_CORES)
    ztb = nc.dram_tensor("ztb", [128, B], BF16, kind="ExternalInput").ap()
    zltb = nc.dram_tensor("zltb", [128, LOCAL], BF16,
                          kind="ExternalInput").ap()
    zl = nc.dram_tensor("zl", [128, LOCAL], FP32, kind="ExternalInput").ap()
    svec = nc.dram_tensor("svec", [128, 2], FP32, kind="ExternalInput").ap()
    outp = nc.dram_tensor("outp", [128, 4 * TILES_I], FP32,
                          kind="ExternalOutput").ap()

    segs = _segments(n1)
    nseg = len(segs)
    cb = sum(1 for s in segs if s[2])
    W_V = CHUNK - W_S

    with tile.TileContext(nc) as tc:
        with (
            tc.tile_pool(name="zp", bufs=1) as zp,
            tc.tile_pool(name="small", bufs=1) as small,
            tc.tile_pool(name="scr", bufs=4) as scrp,
            tc.tile_pool(name="psum", bufs=2, space=bass.MemorySpace.PSUM) as psum,
        ):
            # hoist the Exp table load into the DMA window: a dummy tiny
            # activation first in the scalar queue triggers the (2.7us)
            # ACT_TABLE_LOAD before any real work needs it
            dummy = small.tile([128, 1], FP32, tag="dummy")
            nc.vector.memset(dummy[:], 0.0)
            nc.scalar.activation(dummy[:], dummy[:], AF.Exp)

            # trigger order = HWDGE queue order: svec-path inputs first (they
            # gate the small head-of-queue PE work), then chunk 0, then rest
            zlt = small.tile([128, LOCAL], FP32, tag="zlt")
            nc.sync.dma_start(zlt[:], zl)
            sv = small.tile([128, 2], FP32, tag="sv")
            nc.sync.dma_start(sv[:], svec)
            zlb = small.tile([128, LOCAL], BF16, tag="zlb")
            nc.sync.dma_start(zlb[:], zltb)
            zt = zp.tile([128, B], BF16, tag="zt")
            nc.sync.dma_start(zt[:, 0:CHUNK], ztb[:, 0:CHUNK])
            for q in range(1, NCHUNK):
                for h in range(2):
                    nc.sync.dma_start(
                        zt[:, q * CHUNK + h * 1024:q * CHUNK + (h + 1) * 1024],
                        ztb[:, q * CHUNK + h * 1024:q * CHUNK + (h + 1) * 1024])

            sacc = small.tile([128, TILES_I, nseg], FP32, tag="sacc")
            srr = small.tile([128, 2 * TILES_I], FP32, tag="srr")
            s1r = srr[:, 0:TILES_I]
            s0r = srr[:, TILES_I:2 * TILES_I]

            def chunk_iters(q):
                """One sim chunk: matmuls + split exp row sums, all 8 tiles."""
                qsegs = [(i, s) for i, s in enumerate(segs)
                         if q * CHUNK <= s[0] and s[1] <= (q + 1) * CHUNK]
                v_lo = min(s[0] for _, s in qsegs if s[3] == 'v')
                w_v = CHUNK - _ws_by_chunk(n1)[q]
                for t in range(TILES_I):
                    ps = psum.tile([128, CHUNK], FP32, tag="ps")
                    for h in range(4):
                        nc.tensor.matmul(
                            ps[:, ts(h, 512)], lhsT=zlb[:, ts(t, 128)],
                            rhs=zt[:, q * CHUNK + h * 512:
                                   q * CHUNK + (h + 1) * 512],
                            start=True, stop=True)
                    scr = scrp.tile([128, W_V], INT32, tag="scr")
                    scrf = scr.bitcast(FP32)
                    dead = scrp.tile([128, W_V], FP32, tag="dead")
                    sdead = scrp.tile([128, W_S + 64], BF16, tag="sdead")
                    for si, (lo, hi, _l1, eng) in qsegs:
                        if eng == 's':
                            nc.scalar.activation(
                                sdead[:, 0:hi - lo],
                                ps[:, lo - q * CHUNK:hi - q * CHUNK],
                                AF.Exp, scale=2.0,
                                accum_out=sacc[:, t, si:si + 1])
                    # one affine->int32 pass for the whole DVE range
                    nc.vector.tensor_scalar(
                        out=scr[:, 0:w_v],
                        in0=ps[:, v_lo - q * CHUNK:v_lo - q * CHUNK + w_v],
                        scalar1=SCHRA_A, scalar2=SCHRA_B,
                        op0=ALU.mult, op1=ALU.add)
                    # row sums via tensor_scalar accum
                    for si, (lo, hi, _l1, eng) in qsegs:
                        if eng == 'v':
                            nc.vector.tensor_scalar(
                                out=dead[:, 0:hi - lo],
                                in0=scrf[:, lo - v_lo:hi - v_lo],
                                scalar1=1.0, scalar2=None, op0=ALU.mult,
                                op1=ALU.add,
                                accum_out=sacc[:, t, si:si + 1])

            # tall/t1 = z_loc @ [S, S1] (fp32 exact path for the means) —
            # head of the PE queue, runs during the input DMA window; the
            # raw [tall|t1] block ships to the host mid-loop (the remaining
            # per-row algebra is done host-side in fp64)
            pt = psum.tile([128, CHUNK], FP32, tag="ps")
            for t in range(TILES_I):
                nc.tensor.matmul(pt[:, 2 * t:2 * t + 2],
                                 lhsT=zlt[:, ts(t, 128)], rhs=sv[:],
                                 start=True, stop=True)
            tt = small.tile([128, 2 * TILES_I], FP32, tag="tt")
            nc.vector.tensor_copy(tt[:], pt[:, 0:2 * TILES_I])
            nc.sync.dma_start(outp[:, 0:2 * TILES_I], tt[:])

            chunk_iters(0)

            # remaining chunks; hoist the s1 reduce to right after the last
            # chunk containing label-1 columns so only s0 work is in the tail
            q_l1 = max((s[1] - 1) // CHUNK for s in segs if s[2]) \
                if cb > 0 else -1
            for q in range(1, NCHUNK):
                chunk_iters(q)
                if q == q_l1:
                    nc.vector.reduce_sum(s1r, sacc[:, :, 0:cb], axis=AX.X)
            if q_l1 <= 0:
                if cb > 0:
                    nc.vector.reduce_sum(s1r, sacc[:, :, 0:cb], axis=AX.X)
                else:
                    nc.vector.memset(s1r, 0.0)

            # ---- tail: just the label-0 reduce + raw row-sum shipment ----
            if cb < nseg:
                nc.vector.reduce_sum(s0r, sacc[:, :, cb:nseg], axis=AX.X)
            else:
                nc.vector.memset(s0r, 0.0)
            nc.sync.dma_start(outp[:, 2 * TILES_I:4 * TILES_I], srr[:])

    nc.compile()
    return nc


_NC_CACHE = {}


def _get_nc(n1: int = 4083):
    if n1 not in _NC_CACHE:
        _NC_CACHE[n1] = _build_kernel(n1)
    return _NC_CACHE[n1]


def prepare(feature: np.ndarray, label: np.ndarray):
    """Sort rows by label (1s first), normalize, transpose; per-core maps."""
    feature = np.ascontiguousarray(feature, dtype=np.float32)
    lab = np.asarray(label)
    perm = np.argsort(-lab, kind="stable")
    n1 = int((lab == 1).sum())
    f = feature[perm]
    nrm = np.maximum(np.sqrt((f.astype(np.float64) ** 2).sum(1)), 1e-12)
    z = (f / nrm[:, None]).astype(np.float32)
    zT = np.ascontiguousarray(z.T)                    # [128, B] fp32
    ztb = zT.astype(ml_dtypes.bfloat16)               # [128, B] bf16
    zf = z.astype(np.float64)
    S = zf.sum(0)
    S1 = zf[:n1].sum(0)
    sv = np.ascontiguousarray(
        np.stack([S, S1], axis=1).astype(np.float32))  # [128, 2]
    lsort = lab[perm].astype(np.float64)
    # per-core label matrix in the device's [128, TILES_I] layout
    labm = lsort.reshape(N_CORES, TILES_I, 128).transpose(0, 2, 1)
    in_maps = []
    for c in range(N_CORES):
        sl = slice(c * LOCAL, (c + 1) * LOCAL)
        in_maps.append({
            "ztb": ztb,
            "zltb": np.ascontiguousarray(ztb[:, sl]),
            "zl": np.ascontiguousarray(zT[:, sl]),
            "svec": sv,
        })
    return n1, in_maps, labm


def combine(results, labm):
    """outp = [tall|t1 interleaved (16) | s1r (8) | s0r (8)] per core;
    final per-row algebra and the logs run here in fp64."""
    P = np.stack([np.asarray(r["outp"], dtype=np.float64) for r in results])
    tall = P[:, :, 0:2 * TILES_I:2]
    t1 = P[:, :, 1:2 * TILES_I:2]
    s1 = P[:, :, 2 * TILES_I:3 * TILES_I]
    s0 = P[:, :, 3 * TILES_I:4 * TILES_I]
    pos = labm > 0.5
    same = np.where(pos, s1, s0)
    num = same - EII
    dennum = s1 + s0 - EII
    loss = (np.log(dennum) - np.log(np.maximum(num, 0.0) + EPS)).sum() / B
    same_t = np.where(pos, t1, tall - t1)
    mean_pos = (same_t - 1.0).sum() / (float(B) * B)
    mean_neg = (tall - same_t).sum() / (float(B) * B)
    return (np.float32(loss), np.float32(mean_pos), np.float32(mean_neg))


def run_on_hw(feature, label, **kwargs):
    n1, in_maps, labm = prepare(feature, label)
    nc = _get_nc(n1)
    res = run_bass_kernel_spmd(nc, in_maps,
                               core_ids=list(range(N_CORES)), **kwargs)
    return combine(res.results, labm), res


def kernel(feature: np.ndarray, label: np.ndarray):
    out, _ = run_on_hw(feature, label)
    return out
